# revision 83
# baseline (speedup 1.0000x reference)
"""Trainium2 Bass kernel for DecoderAttention (b=2, n=2048, m=1024, d=1024, h=16).

Sharding: 8 cores = 2 (batch) x 4 (head groups of 4 heads).  Each core:
  - projects q/k/v for its 4 heads from x|context (pre-transposed on host),
  - runs causal flash attention in scores-transposed layout [kj, qi]
    (softmax without max subtraction -- scores are bounded; causally masked
    entries multiply to exactly 0 after exp, matching exp(-50000)),
  - computes its partial out-projection  attn_out_g @ Wo[rows_g]  [2048, 1024]
    emitted as bf16.
Host sums the 4 head-group partials per batch (the "all-reduce") and adds bo.

All matmuls run in bf16 with f32 PSUM accumulation (~0.4% rel err; fp8 was
measured at 5-7% end-to-end on the host and is ruled out).

Schedule notes (measured on HW, 354us -> ~253us):
  - The attention body is TensorE-bound at ~1.4us per key-tile round:
    score pair (row-tiled 64x128, truly concurrent: one qT stream feeds
    both sub-arrays) + AV pair (column-tiled 128x64; serialized anyway
    because both heads stream different pt columns through the one rhs
    port) + ~2 drip-fed projection matmuls as filler.
  - exp on ScalarE (144 x ~1.1us) is the close second; scores psum is
    double-buffered so exps run back-to-back except where the PE falls
    behind.
  - The softmax denominator is accumulated elementwise on DVE in bf16 and
    reduced+broadcast by one column-tiled ones-matmul pair per head-pair,
    then reciprocal_approx_fast (the precise DVE reciprocal costs 3.3us
    per tile and had been 53us of the baseline).  GpSimd offload of the
    accumulation poisons SBUF ports (-25% on every engine); SWDGE
    dma-accumulate serializes on its completion chain (2x slower overall).
  - Input DMAs are issued in consumption order (q/k weights, first query
    chunk columns, context columns, rest) and dummy matmuls warm the HAM
    clock gate (1.2 -> 2.4 GHz) during the DMA wait; more dummies keep it
    warm into the final out-projection.
  - Diagonal-tile score/AV matmuls skip the fully-masked leading query
    columns (128*dt) for chunks > 0; chunk 0 streams full width because
    its first scores-psum use would otherwise expose uninitialized PSUM
    to the exp (NaN * mask-zero = NaN).
"""

import os

# The neuron/axon jax backend must be discoverable for the PJRT execution
# path; a JAX_PLATFORMS=cpu pin (used when running the jax reference) would
# hide the trn2 devices from this process.
if os.environ.get("JAX_PLATFORMS", "").strip().lower() == "cpu":
    del os.environ["JAX_PLATFORMS"]

import itertools
from contextlib import ExitStack

import ml_dtypes
import numpy as np

import concourse.bass as bass
import concourse.tile as tile
from concourse import bacc, mybir
from concourse.bass_utils import run_bass_kernel_spmd

B, N, M, D = 2, 2048, 1024, 1024
H, DH = 16, 64
NM = N + M          # 3072 keys (self + context)
GROUPS = 4          # head groups; 4 heads = 256 cols per group
GC = 256            # columns per head group
NCORES = 8
SCALE = DH ** -0.5
P = 128
KT = D // P         # 8 contraction tiles over d
QCH = 512           # query-chunk width
NQC = N // QCH      # 4 query chunks
NKJ = NM // P       # 24 key tiles
NSELF = N // P      # 16 self key tiles
FP32 = mybir.dt.float32
F32R = mybir.dt.float32r
BF16 = mybir.dt.bfloat16
BF16NP = ml_dtypes.bfloat16


def _active_kj(c):
    """Key tiles with any unmasked entry for query chunk c (512 queries)."""
    return list(range(0, 4 * c + 4)) + list(range(NSELF, NKJ))


def _build_module(biased: bool):
    nc = bacc.Bacc(
        "TRN2",
        target_bir_lowering=False,
        debug=False,
        enable_asserts=False,
        num_devices=NCORES,
    )
    xkvT_d = nc.dram_tensor("xkvT", [D, NM], BF16, kind="ExternalInput").ap()
    wq_d = nc.dram_tensor("wq", [D, GC], BF16, kind="ExternalInput").ap()
    wk_d = nc.dram_tensor("wk", [D, GC], BF16, kind="ExternalInput").ap()
    wv_d = nc.dram_tensor("wv", [D, GC], BF16, kind="ExternalInput").ap()
    wo_d = nc.dram_tensor("wo", [GC, D], BF16, kind="ExternalInput").ap()
    msk_d = nc.dram_tensor("msk", [4 * P, QCH], BF16, kind="ExternalInput").ap()
    if biased:
        bq_d = nc.dram_tensor("bq", [1, GC], BF16, kind="ExternalInput").ap()
        bk_d = nc.dram_tensor("bk", [1, GC], BF16, kind="ExternalInput").ap()
        bv_d = nc.dram_tensor("bv", [1, GC], BF16, kind="ExternalInput").ap()
    out_d = nc.dram_tensor("out", [N, D], BF16, kind="ExternalOutput").ap()

    with tile.TileContext(nc) as tc, ExitStack() as ctx:
        const = ctx.enter_context(tc.tile_pool(name="const", bufs=1))
        pexp = ctx.enter_context(tc.tile_pool(name="pexp", bufs=12))
        bcp = ctx.enter_context(tc.tile_pool(name="bcp", bufs=3))
        # PSUM budget: 8 banks = proj(1) + bc(1) + scores(2x2) + av(2)
        ps_main = ctx.enter_context(tc.tile_pool(name="ps_main", bufs=1, space="PSUM"))
        ps_s = ctx.enter_context(tc.tile_pool(name="ps_s", bufs=2, space="PSUM"))
        ps_av = ctx.enter_context(tc.tile_pool(name="ps_av", bufs=2, space="PSUM"))

        # ---- persistent SBUF tensors (column-concatenated k-tiles) ----
        xk = const.tile([P, KT * NM], BF16)          # xkvT: 8 tiles of [128, 3072]
        wqs = const.tile([P, KT * GC], BF16)
        wks = const.tile([P, KT * GC], BF16)
        wvs = const.tile([P, KT * GC], BF16)
        wos = const.tile([P, 2 * D], BF16)           # Wo rows: 2 tiles of [128, 1024]
        mks = const.tile([P, 4 * QCH], BF16)         # 4 diagonal mask tiles
        qT = const.tile([P, 2 * N], BF16)            # [head-pair cols, qi]
        kT = const.tile([P, 2 * NM], BF16)           # [head-pair cols, kj]
        vv = const.tile([P, NKJ * GC], BF16)         # per kj tile: 4 heads x 64
        aT = const.tile([P, 2 * N], BF16)            # attn_out^T, 2 k-tiles
        ones_bc = const.tile([P, 64], BF16)          # all-ones: den reduce+broadcast
        if biased:
            bq_s = const.tile([1, GC], BF16)
            bk_s = const.tile([1, GC], BF16)
            bv_s = const.tile([1, GC], BF16)
            ones_row = const.tile([1, QCH], BF16)
            ones_col = const.tile([1, P], BF16)

        # ---- input DMAs ----
        # One batched DMA per tensor/column-chunk, ordered so the first
        # projections (weights, then x columns for query-chunk 0, then the
        # context columns) unblock compute within a few us instead of after
        # the whole ~9 MB input load.
        xk_v = xk.rearrange("p (kt m) -> p kt m", kt=KT)
        xkvT_v = xkvT_d.rearrange("(kt p) m -> p kt m", p=P)

        def dma_xk(cc):
            nc.sync.dma_start(
                xk_v[:, :, cc * QCH:(cc + 1) * QCH],
                xkvT_v[:, :, cc * QCH:(cc + 1) * QCH],
            )

        # Transfers complete in issue order, so the sequence below is the
        # critical-path priority: q/k weights and the first query-chunk's
        # x columns gate the first scores, then the cross-attention
        # columns, then the rest.  (Finer-grained per-k-tile splits and
        # deferred-injection prologues were tried and measured worse: the
        # wv/mks transfers just land later and stall the first AV/mask
        # rounds instead.)
        nc.sync.dma_start(
            wqs.rearrange("p (kt g) -> p kt g", kt=KT),
            wq_d.rearrange("(kt p) g -> p kt g", p=P),
        )
        nc.sync.dma_start(
            wks.rearrange("p (kt g) -> p kt g", kt=KT),
            wk_d.rearrange("(kt p) g -> p kt g", p=P),
        )
        dma_xk(0)
        dma_xk(4)
        dma_xk(5)
        nc.sync.dma_start(
            wvs.rearrange("p (kt g) -> p kt g", kt=KT),
            wv_d.rearrange("(kt p) g -> p kt g", p=P),
        )
        nc.sync.dma_start(
            mks.rearrange("p (t q) -> p t q", t=4),
            msk_d.rearrange("(t p) q -> p t q", p=P),
        )
        for cc in (1, 2, 3):
            dma_xk(cc)
        nc.sync.dma_start(
            wos.rearrange("p (t d) -> p t d", t=2),
            wo_d.rearrange("(t p) d -> p t d", p=P),
        )
        nc.vector.memset(ones_bc[:], 1.0)
        if biased:
            nc.sync.dma_start(bq_s[:], bq_d[:])
            nc.sync.dma_start(bk_s[:], bk_d[:])
            nc.sync.dma_start(bv_s[:], bv_d[:])
            nc.vector.memset(ones_row[:], 1.0)
            nc.vector.memset(ones_col[:], 1.0)

        # ---- PE warm-up: the HAM clock gate keeps the PE at 1.2 GHz until
        # it has been busy ~3.4us.  Dummy matmuls during the input-DMA wait
        # bring it to 2.4 GHz before the first real projection chain.
        wrm = const.tile([P, QCH], BF16)
        nc.vector.memset(wrm[:], 1.0)
        ps_w = ps_main.tile([P, QCH], FP32, tag="proj", name="warm")
        for _ in range(30):
            nc.tensor.matmul(
                ps_w[:], lhsT=wrm[:, 0:P], rhs=wrm[:], start=True, stop=True,
            )

        # ---- emission helpers (generators yield every ~2 matmuls so proj
        # work can be drip-fed into the PE queue between attention rounds
        # without delaying the score matmuls that feed the exp pipeline) ----
        def gen_qT_group(mt, c):
            psq = ps_main.tile([P, QCH], FP32, tag="proj", name="psq")
            for kt in range(KT):
                nc.tensor.matmul(
                    psq[:],
                    lhsT=wqs[:, kt * GC + mt * P: kt * GC + (mt + 1) * P],
                    rhs=xk[:, kt * NM + c * QCH: kt * NM + (c + 1) * QCH],
                    start=(kt == 0),
                    stop=(kt == KT - 1) and not biased,
                )
                if kt % 2 == 1:
                    yield
            if biased:
                nc.tensor.matmul(
                    psq[:], lhsT=bq_s[:, mt * P:(mt + 1) * P], rhs=ones_row[:],
                    start=False, stop=True,
                )
            nc.vector.tensor_copy(
                qT[:, mt * N + c * QCH: mt * N + (c + 1) * QCH], psq[:]
            )
            yield

        def gen_kT_group(mt, c2):
            psk = ps_main.tile([P, QCH], FP32, tag="proj", name="psk")
            for kt in range(KT):
                nc.tensor.matmul(
                    psk[:],
                    lhsT=wks[:, kt * GC + mt * P: kt * GC + (mt + 1) * P],
                    rhs=xk[:, kt * NM + c2 * QCH: kt * NM + (c2 + 1) * QCH],
                    start=(kt == 0),
                    stop=(kt == KT - 1) and not biased,
                )
                if kt % 2 == 1:
                    yield
            if biased:
                nc.tensor.matmul(
                    psk[:], lhsT=bk_s[:, mt * P:(mt + 1) * P], rhs=ones_row[:],
                    start=False, stop=True,
                )
            nc.vector.tensor_copy(
                kT[:, mt * NM + c2 * QCH: mt * NM + (c2 + 1) * QCH], psk[:]
            )
            yield

        def gen_v_pair(t0):
            # two key-tiles' V side by side in one psum bank -> one evict.
            # start=True only on the very first matmul: its whole-bank
            # has_written clear covers both halves (same engine, serial).
            psv = ps_main.tile([P, 2 * GC], FP32, tag="proj", name="psv")
            for kt in range(KT):
                for j in range(2):
                    nc.tensor.matmul(
                        psv[:, j * GC:(j + 1) * GC],
                        lhsT=xk[:, kt * NM + (t0 + j) * P: kt * NM + (t0 + j + 1) * P],
                        rhs=wvs[:, kt * GC:(kt + 1) * GC],
                        start=(kt == 0 and j == 0),
                        stop=(kt == KT - 1) and not biased,
                    )
                yield
            if biased:
                for j in range(2):
                    nc.tensor.matmul(
                        psv[:, j * GC:(j + 1) * GC], lhsT=ones_col[:], rhs=bv_s[:],
                        start=False, stop=True,
                    )
            nc.vector.tensor_copy(vv[:, t0 * GC:(t0 + 2) * GC], psv[:])
            yield

        def gen_outproj_chunk(c, tail=False):
            for it in range(4 * c, 4 * c + 4):
                for nh in range(2):
                    pso = ps_main.tile([P, QCH], FP32, tag="proj", name="pso")
                    for kt in range(2):
                        nc.tensor.matmul(
                            pso[:],
                            lhsT=aT[:, kt * N + it * P: kt * N + (it + 1) * P],
                            rhs=wos[:, kt * D + nh * QCH: kt * D + (nh + 1) * QCH],
                            start=(kt == 0),
                            stop=(kt == 1),
                        )
                    osb = pexp.tile([P, QCH], BF16, tag="osb", bufs=3, name="osb")
                    # bf16 eviction halves both the copy and the output DMA;
                    # in the drained tail (no exps in flight) ACT shares it.
                    if tail and nh == 0:
                        nc.scalar.copy(osb[:], pso[:])
                    else:
                        nc.vector.tensor_copy(osb[:], pso[:])
                    nc.sync.dma_start(
                        out_d[it * P:(it + 1) * P, nh * QCH:(nh + 1) * QCH], osb[:]
                    )
                    yield

        def drain(g):
            for _ in g:
                pass

        # normalize: the denominator arrives for free in AV row 64 (ones
        # column of vv).  Per head: evict it, approx-reciprocal, broadcast
        # to 64 partitions via a tiny matmul (bf16 view of the fp32
        # reciprocal -- high half-words -- keeps the matmul off the 4-pass
        # fp32 path), then one fused PSUM-read multiply into aT.
        def emit_norm(ps_acc, den_acc, c, pair):
            dbc = ps_main.tile([P, QCH], FP32, tag="bc", name="dbc")
            for hh in range(2):
                lo = hh * 64
                nc.tensor.matmul(
                    dbc[lo:lo + 64, :],
                    lhsT=ones_bc[:, 0:64],
                    rhs=den_acc[:, hh * QCH:(hh + 1) * QCH],
                    start=True,
                    stop=True,
                )
            dbs = bcp.tile([P, QCH], FP32, tag="dbs", name="dbs")
            nc.vector.tensor_copy(dbs[:], dbc[:])
            rbc = bcp.tile([P, QCH], FP32, tag="rbc", name="rbc")
            nc.vector.reciprocal_approx_fast(rbc[:], dbs[:])
            for hh in range(2):
                lo = hh * 64
                nc.vector.tensor_mul(
                    aT[lo:lo + 64, pair * N + c * QCH: pair * N + (c + 1) * QCH],
                    ps_acc[hh][lo:lo + 64, :],
                    rbc[lo:lo + 64, :],
                )

        def emit_attention_chunk(c, filler=None, warm_tail=False):
            kjs = _active_kj(c)
            last = len(kjs) - 1
            for pair in range(2):
                ps_acc = [None, None]
                den_acc = None
                pending = []  # (p_tile, i) exp'd tiles not yet fed to AV

                def trim(t):
                    # diagonal tile dt: query columns < 128*dt are fully
                    # causally masked -- skip streaming them.  Not for
                    # chunk 0, whose first scores-psum use would otherwise
                    # leave uninitialized PSUM under the exp (NaN*0=NaN).
                    if c > 0 and 4 * c <= t < 4 * c + 4:
                        return P * (t - 4 * c)
                    return 0

                def do_av(pt, i):
                    # both heads via 128x64 column tiling: head hh lands on
                    # PSUM partitions hh*64..hh*64+63 of its own bank
                    # (separate banks -- the whole-bank has_written clear of
                    # start=True must not race the other head's
                    # accumulation).
                    t = kjs[i]
                    off = trim(t)
                    for hh in range(2):
                        h = pair * 2 + hh
                        lo = hh * 64
                        nc.tensor.matmul(
                            ps_acc[hh][lo:lo + 64, off:QCH],
                            lhsT=vv[:, t * GC + h * 64: t * GC + (h + 1) * 64],
                            rhs=pt[:, hh * QCH + off:(hh + 1) * QCH],
                            start=(i == 0),
                            stop=(i == last),
                        )

                for i, t in enumerate(kjs):
                    # both heads' scores into one 2-bank psum tile
                    off = trim(t)
                    pss = ps_s.tile([P, 2 * QCH], FP32, tag="s", name="pss")
                    for hh in range(2):
                        lo, hi = hh * 64, hh * 64 + 64
                        nc.tensor.matmul(
                            pss[:, hh * QCH + off:(hh + 1) * QCH],
                            lhsT=kT[lo:hi, pair * NM + t * P: pair * NM + (t + 1) * P],
                            rhs=qT[lo:hi, pair * N + c * QCH + off:
                                   pair * N + (c + 1) * QCH],
                            start=True,
                            stop=True,
                        )
                    pt = pexp.tile([P, 2 * QCH], BF16, tag="p", name="pt")
                    nc.scalar.activation(
                        pt[:], pss[:], mybir.ActivationFunctionType.Exp
                    )
                    if 4 * c <= t < 4 * c + 4:  # diagonal tile: causal mask
                        dt = t - 4 * c
                        for hh in range(2):
                            nc.vector.tensor_mul(
                                pt[:, hh * QCH:(hh + 1) * QCH],
                                pt[:, hh * QCH:(hh + 1) * QCH],
                                mks[:, dt * QCH:(dt + 1) * QCH],
                            )
                    # softmax denominator: elementwise accumulate the exp'd
                    # tiles on DVE (the cross-key reduction happens in one
                    # reduce+broadcast matmul per pair at pair end).
                    # Offloading this to GpSimd compute contends for SBUF
                    # ports and slows every engine ~25%; SWDGE dma-accum
                    # serializes on its completion chain; the M=65 ones-row
                    # variant was also measured slower end-to-end.
                    if i == 0:
                        den_acc = bcp.tile(
                            [P, 2 * QCH], BF16, tag="dacc", bufs=2, name="dacc"
                        )
                        nc.vector.tensor_copy(den_acc[:], pt[:])
                    else:
                        nc.vector.tensor_add(den_acc[:], den_acc[:], pt[:])
                    pending.append((pt, i))
                    if i == 0:
                        ps_acc[0] = ps_av.tile([P, QCH], FP32, tag="av", name="av0")
                        ps_acc[1] = ps_av.tile([P, QCH], FP32, tag="av", name="av1")
                    while len(pending) > 1:
                        do_av(*pending.pop(0))
                    if filler is not None:
                        next(filler, None)
                while pending:
                    do_av(*pending.pop(0))
                if warm_tail and pair == 1:
                    # the final den-reduce waits ~3us on the DVE denominator
                    # backlog; dependency-free dummies ahead of it in the
                    # FIFO keep the PE busy through that wait (HAM would
                    # re-throttle to 1.2 GHz after ~3.4us idle and the
                    # whole final out-projection would run at half clock)
                    for _ in range(16):
                        nc.tensor.matmul(
                            ps_w[:], lhsT=wrm[:, 0:P], rhs=wrm[:],
                            start=True, stop=True,
                        )
                emit_norm(ps_acc, den_acc, c, pair)

        # ---- emission: prologue projections ordered so pair-0's scores
        # unblock first (mt=0 q/k), then attention chunks with the next
        # chunk's projections and the previous chunk's out-projection
        # drip-fed into the PE queue as filler between attention rounds ----
        drain(gen_qT_group(0, 0))
        drain(gen_kT_group(0, 0))
        drain(gen_qT_group(1, 0))
        drain(gen_kT_group(1, 0))
        drain(gen_v_pair(0))
        drain(gen_v_pair(2))
        for c2 in (4, 5):
            for mt in range(2):
                drain(gen_kT_group(mt, c2))
        for t0 in range(NSELF, NKJ, 2):
            drain(gen_v_pair(t0))
        for c in range(NQC):
            work = []
            if c < NQC - 1:
                c1 = c + 1
                work += [
                    gen_qT_group(0, c1), gen_kT_group(0, c1),
                    gen_qT_group(1, c1), gen_kT_group(1, c1),
                    gen_v_pair(4 * c1), gen_v_pair(4 * c1 + 2),
                ]
            if c > 0:
                work.append(gen_outproj_chunk(c - 1))
            filler = itertools.chain(*work)
            emit_attention_chunk(c, filler, warm_tail=(c == NQC - 1))
            drain(filler)
        # a few more dummies cover the normalize->aT wait just before the
        # final out-projection
        for _ in range(8):
            nc.tensor.matmul(
                ps_w[:], lhsT=wrm[:, 0:P], rhs=wrm[:], start=True, stop=True,
            )
        drain(gen_outproj_chunk(NQC - 1, tail=True))

    nc.compile()
    return nc


_CACHE: dict = {}


def _module(biased: bool):
    if biased not in _CACHE:
        _CACHE[biased] = _build_module(biased)
    return _CACHE[biased]


def _mask_tiles():
    t = np.arange(4)[:, None, None]
    p = np.arange(P)[None, :, None]
    q = np.arange(QCH)[None, None, :]
    return (p + P * t <= q).astype(BF16NP).reshape(4 * P, QCH)


def kernel(x, context, Wq, bq, Wkv, bkv, Wo, bo, mask, context_mask):
    assert bool(np.all(mask)) and bool(np.all(context_mask)), (
        "only all-true padding masks are supported"
    )
    x = np.asarray(x, np.float32)
    context = np.asarray(context, np.float32)
    Wq, bq = np.asarray(Wq, np.float32), np.asarray(bq, np.float32)
    Wkv, bkv = np.asarray(Wkv, np.float32), np.asarray(bkv, np.float32)
    Wo, bo = np.asarray(Wo, np.float32), np.asarray(bo, np.float32)

    biased = bool(np.any(bq) or np.any(bkv))
    nc = _module(biased)

    msk = _mask_tiles()
    xkvT = [
        np.ascontiguousarray(
            np.concatenate([x[b], context[b]], axis=0).T.astype(BF16NP)
        )
        for b in range(B)
    ]
    in_maps = []
    for core in range(NCORES):
        b, g = divmod(core, GROUPS)
        cols = slice(g * GC, (g + 1) * GC)
        im = {
            "xkvT": xkvT[b],
            "wq": (Wq[:, cols] * SCALE).astype(BF16NP),
            "wk": Wkv[:, cols].astype(BF16NP),
            "wv": Wkv[:, D + g * GC: D + (g + 1) * GC].astype(BF16NP),
            "wo": np.ascontiguousarray(Wo[cols, :]).astype(BF16NP),
            "msk": msk,
        }
        if biased:
            im["bq"] = (bq[cols] * SCALE).astype(BF16NP).reshape(1, GC)
            im["bk"] = bkv[cols].astype(BF16NP).reshape(1, GC)
            im["bv"] = bkv[D + g * GC: D + (g + 1) * GC].astype(BF16NP).reshape(1, GC)
        in_maps.append(im)

    try:
        res = run_bass_kernel_spmd(nc, in_maps, core_ids=list(range(NCORES)))
    except ModuleNotFoundError:
        # BASS_TRACE set but the NTFF profiling hook isn't available in this
        # environment -- rerun with tracing hard-disabled.
        os.environ["BASS_NEVER_TRACE"] = "1"
        res = run_bass_kernel_spmd(nc, in_maps, core_ids=list(range(NCORES)))
    kernel.last_results = res
    out = np.zeros((B, N, D), np.float32)
    for core in range(NCORES):
        b = core // GROUPS
        out[b] += np.asarray(res.results[core]["out"], dtype=np.float32)
    out += bo
    return out



# revision 86
# speedup vs baseline: 1.0140x; 1.0140x over previous
"""Trainium2 Bass kernel for DecoderAttention (b=2, n=2048, m=1024, d=1024, h=16).

Sharding: 8 cores = 2 (batch) x 4 (head groups of 4 heads).  Each core:
  - projects q/k/v for its 4 heads from x|context (pre-transposed on host),
  - runs causal flash attention in scores-transposed layout [kj, qi]
    (softmax without max subtraction -- scores are bounded; causally masked
    entries multiply to exactly 0 after exp, matching exp(-50000)),
  - computes its partial out-projection  attn_out_g @ Wo[rows_g]  [2048, 1024]
    emitted as bf16.
Host sums the 4 head-group partials per batch (the "all-reduce") and adds bo.

All matmuls run in bf16 with f32 PSUM accumulation (~0.4% rel err; fp8 was
measured at 5-7% end-to-end on the host and is ruled out).

Schedule notes (measured on HW, 354us -> ~253us):
  - The attention body is TensorE-bound at ~1.4us per key-tile round:
    score pair (row-tiled 64x128, truly concurrent: one qT stream feeds
    both sub-arrays) + AV pair (column-tiled 128x64; serialized anyway
    because both heads stream different pt columns through the one rhs
    port) + ~2 drip-fed projection matmuls as filler.
  - exp on ScalarE (144 x ~1.1us) is the close second; scores psum is
    double-buffered so exps run back-to-back except where the PE falls
    behind.
  - The softmax denominator is accumulated elementwise on DVE in bf16 and
    reduced+broadcast by one column-tiled ones-matmul pair per head-pair,
    then reciprocal_approx_fast (the precise DVE reciprocal costs 3.3us
    per tile and had been 53us of the baseline).  GpSimd offload of the
    accumulation poisons SBUF ports (-25% on every engine); SWDGE
    dma-accumulate serializes on its completion chain (2x slower overall).
  - Input DMAs are issued in consumption order (q/k weights, first query
    chunk columns, context columns, rest) and dummy matmuls warm the HAM
    clock gate (1.2 -> 2.4 GHz) during the DMA wait; more dummies keep it
    warm into the final out-projection.
  - Diagonal-tile score/AV matmuls skip the fully-masked leading query
    columns (128*dt) for chunks > 0; chunk 0 streams full width because
    its first scores-psum use would otherwise expose uninitialized PSUM
    to the exp (NaN * mask-zero = NaN).
"""

import os

# The neuron/axon jax backend must be discoverable for the PJRT execution
# path; a JAX_PLATFORMS=cpu pin (used when running the jax reference) would
# hide the trn2 devices from this process.
if os.environ.get("JAX_PLATFORMS", "").strip().lower() == "cpu":
    del os.environ["JAX_PLATFORMS"]

import itertools
from contextlib import ExitStack

import ml_dtypes
import numpy as np

import concourse.bass as bass
import concourse.tile as tile
from concourse import bacc, mybir
from concourse.bass_utils import run_bass_kernel_spmd

B, N, M, D = 2, 2048, 1024, 1024
H, DH = 16, 64
NM = N + M          # 3072 keys (self + context)
GROUPS = 4          # head groups; 4 heads = 256 cols per group
GC = 256            # columns per head group
NCORES = 8
SCALE = DH ** -0.5
P = 128
KT = D // P         # 8 contraction tiles over d
QCH = 512           # query-chunk width
NQC = N // QCH      # 4 query chunks
NKJ = NM // P       # 24 key tiles
NSELF = N // P      # 16 self key tiles
FP32 = mybir.dt.float32
F32R = mybir.dt.float32r
BF16 = mybir.dt.bfloat16
BF16NP = ml_dtypes.bfloat16


def _active_kj(c):
    """Key tiles with any unmasked entry for query chunk c (512 queries)."""
    return list(range(0, 4 * c + 4)) + list(range(NSELF, NKJ))


def _build_module(biased: bool):
    nc = bacc.Bacc(
        "TRN2",
        target_bir_lowering=False,
        debug=False,
        enable_asserts=False,
        num_devices=NCORES,
    )
    xkvT_d = nc.dram_tensor("xkvT", [D, NM], BF16, kind="ExternalInput").ap()
    wq_d = nc.dram_tensor("wq", [D, GC], BF16, kind="ExternalInput").ap()
    wk_d = nc.dram_tensor("wk", [D, GC], BF16, kind="ExternalInput").ap()
    wv_d = nc.dram_tensor("wv", [D, GC], BF16, kind="ExternalInput").ap()
    wo_d = nc.dram_tensor("wo", [GC, D], BF16, kind="ExternalInput").ap()
    msk_d = nc.dram_tensor("msk", [4 * P, QCH], BF16, kind="ExternalInput").ap()
    if biased:
        bq_d = nc.dram_tensor("bq", [1, GC], BF16, kind="ExternalInput").ap()
        bk_d = nc.dram_tensor("bk", [1, GC], BF16, kind="ExternalInput").ap()
        bv_d = nc.dram_tensor("bv", [1, GC], BF16, kind="ExternalInput").ap()
    out_d = nc.dram_tensor("out", [N, D], BF16, kind="ExternalOutput").ap()

    with tile.TileContext(nc) as tc, ExitStack() as ctx:
        const = ctx.enter_context(tc.tile_pool(name="const", bufs=1))
        pexp = ctx.enter_context(tc.tile_pool(name="pexp", bufs=12))
        bcp = ctx.enter_context(tc.tile_pool(name="bcp", bufs=3))
        # PSUM budget: 8 banks = proj(1) + bc(1) + scores(2x2) + av(2)
        ps_main = ctx.enter_context(tc.tile_pool(name="ps_main", bufs=1, space="PSUM"))
        ps_s = ctx.enter_context(tc.tile_pool(name="ps_s", bufs=2, space="PSUM"))
        ps_av = ctx.enter_context(tc.tile_pool(name="ps_av", bufs=2, space="PSUM"))

        # ---- persistent SBUF tensors (column-concatenated k-tiles) ----
        xk = const.tile([P, KT * NM], BF16)          # xkvT: 8 tiles of [128, 3072]
        wqs = const.tile([P, KT * GC], BF16)
        wks = const.tile([P, KT * GC], BF16)
        wvs = const.tile([P, KT * GC], BF16)
        wos = const.tile([P, 2 * D], BF16)           # Wo rows: 2 tiles of [128, 1024]
        mks = const.tile([P, 4 * QCH], BF16)         # 4 diagonal mask tiles
        qT = const.tile([P, 2 * N], BF16)            # [head-pair cols, qi]
        kT = const.tile([P, 2 * NM], BF16)           # [head-pair cols, kj]
        vv = const.tile([P, NKJ * GC], BF16)         # per kj tile: 4 heads x 64
        aT = const.tile([P, 2 * N], BF16)            # attn_out^T, 2 k-tiles
        ones_bc = const.tile([P, 64], BF16)          # all-ones: den reduce+broadcast
        if biased:
            bq_s = const.tile([1, GC], BF16)
            bk_s = const.tile([1, GC], BF16)
            bv_s = const.tile([1, GC], BF16)
            ones_row = const.tile([1, QCH], BF16)
            ones_col = const.tile([1, P], BF16)

        # ---- input DMAs ----
        # One batched DMA per tensor/column-chunk, ordered so the first
        # projections (weights, then x columns for query-chunk 0, then the
        # context columns) unblock compute within a few us instead of after
        # the whole ~9 MB input load.
        xk_v = xk.rearrange("p (kt m) -> p kt m", kt=KT)
        xkvT_v = xkvT_d.rearrange("(kt p) m -> p kt m", p=P)

        def dma_xk(cc):
            nc.sync.dma_start(
                xk_v[:, :, cc * QCH:(cc + 1) * QCH],
                xkvT_v[:, :, cc * QCH:(cc + 1) * QCH],
            )

        # Transfers complete in issue order, so the sequence below is the
        # critical-path priority: q/k weights and the first query-chunk's
        # x columns gate the first scores, then the cross-attention
        # columns, then the rest.  (Finer-grained per-k-tile splits and
        # deferred-injection prologues were tried and measured worse: the
        # wv/mks transfers just land later and stall the first AV/mask
        # rounds instead.)
        nc.sync.dma_start(
            wqs.rearrange("p (kt g) -> p kt g", kt=KT),
            wq_d.rearrange("(kt p) g -> p kt g", p=P),
        )
        nc.sync.dma_start(
            wks.rearrange("p (kt g) -> p kt g", kt=KT),
            wk_d.rearrange("(kt p) g -> p kt g", p=P),
        )
        dma_xk(0)
        dma_xk(4)
        dma_xk(5)
        nc.sync.dma_start(
            wvs.rearrange("p (kt g) -> p kt g", kt=KT),
            wv_d.rearrange("(kt p) g -> p kt g", p=P),
        )
        nc.sync.dma_start(
            mks.rearrange("p (t q) -> p t q", t=4),
            msk_d.rearrange("(t p) q -> p t q", p=P),
        )
        for cc in (1, 2, 3):
            dma_xk(cc)
        nc.sync.dma_start(
            wos.rearrange("p (t d) -> p t d", t=2),
            wo_d.rearrange("(t p) d -> p t d", p=P),
        )
        nc.vector.memset(ones_bc[:], 1.0)
        if biased:
            nc.sync.dma_start(bq_s[:], bq_d[:])
            nc.sync.dma_start(bk_s[:], bk_d[:])
            nc.sync.dma_start(bv_s[:], bv_d[:])
            nc.vector.memset(ones_row[:], 1.0)
            nc.vector.memset(ones_col[:], 1.0)

        # ---- PE warm-up: the HAM clock gate keeps the PE at 1.2 GHz until
        # it has been busy ~3.4us.  Dummy matmuls during the input-DMA wait
        # bring it to 2.4 GHz before the first real projection chain.
        wrm = const.tile([P, QCH], BF16)
        nc.vector.memset(wrm[:], 1.0)
        ps_w = ps_main.tile([P, QCH], FP32, tag="proj", name="warm")
        for _ in range(30):
            nc.tensor.matmul(
                ps_w[:], lhsT=wrm[:, 0:P], rhs=wrm[:], start=True, stop=True,
            )

        # ---- emission helpers (generators yield every ~2 matmuls so proj
        # work can be drip-fed into the PE queue between attention rounds
        # without delaying the score matmuls that feed the exp pipeline) ----
        def gen_qT_group(mt, c):
            psq = ps_main.tile([P, QCH], FP32, tag="proj", name="psq")
            for kt in range(KT):
                nc.tensor.matmul(
                    psq[:],
                    lhsT=wqs[:, kt * GC + mt * P: kt * GC + (mt + 1) * P],
                    rhs=xk[:, kt * NM + c * QCH: kt * NM + (c + 1) * QCH],
                    start=(kt == 0),
                    stop=(kt == KT - 1) and not biased,
                )
                if kt % 2 == 1:
                    yield
            if biased:
                nc.tensor.matmul(
                    psq[:], lhsT=bq_s[:, mt * P:(mt + 1) * P], rhs=ones_row[:],
                    start=False, stop=True,
                )
            nc.vector.tensor_copy(
                qT[:, mt * N + c * QCH: mt * N + (c + 1) * QCH], psq[:]
            )
            yield

        def gen_kT_group(mt, c2):
            psk = ps_main.tile([P, QCH], FP32, tag="proj", name="psk")
            for kt in range(KT):
                nc.tensor.matmul(
                    psk[:],
                    lhsT=wks[:, kt * GC + mt * P: kt * GC + (mt + 1) * P],
                    rhs=xk[:, kt * NM + c2 * QCH: kt * NM + (c2 + 1) * QCH],
                    start=(kt == 0),
                    stop=(kt == KT - 1) and not biased,
                )
                if kt % 2 == 1:
                    yield
            if biased:
                nc.tensor.matmul(
                    psk[:], lhsT=bk_s[:, mt * P:(mt + 1) * P], rhs=ones_row[:],
                    start=False, stop=True,
                )
            nc.vector.tensor_copy(
                kT[:, mt * NM + c2 * QCH: mt * NM + (c2 + 1) * QCH], psk[:]
            )
            yield

        def gen_v_pair(t0):
            # two key-tiles' V side by side in one psum bank -> one evict.
            # start=True only on the very first matmul: its whole-bank
            # has_written clear covers both halves (same engine, serial).
            psv = ps_main.tile([P, 2 * GC], FP32, tag="proj", name="psv")
            for kt in range(KT):
                for j in range(2):
                    nc.tensor.matmul(
                        psv[:, j * GC:(j + 1) * GC],
                        lhsT=xk[:, kt * NM + (t0 + j) * P: kt * NM + (t0 + j + 1) * P],
                        rhs=wvs[:, kt * GC:(kt + 1) * GC],
                        start=(kt == 0 and j == 0),
                        stop=(kt == KT - 1) and not biased,
                    )
                yield
            if biased:
                for j in range(2):
                    nc.tensor.matmul(
                        psv[:, j * GC:(j + 1) * GC], lhsT=ones_col[:], rhs=bv_s[:],
                        start=False, stop=True,
                    )
            nc.vector.tensor_copy(vv[:, t0 * GC:(t0 + 2) * GC], psv[:])
            yield

        def gen_outproj_chunk(c, tail=False):
            for it in range(4 * c, 4 * c + 4):
                for nh in range(2):
                    pso = ps_main.tile([P, QCH], FP32, tag="proj", name="pso")
                    for kt in range(2):
                        nc.tensor.matmul(
                            pso[:],
                            lhsT=aT[:, kt * N + it * P: kt * N + (it + 1) * P],
                            rhs=wos[:, kt * D + nh * QCH: kt * D + (nh + 1) * QCH],
                            start=(kt == 0),
                            stop=(kt == 1),
                        )
                    osb = pexp.tile([P, QCH], BF16, tag="osb", bufs=3, name="osb")
                    # bf16 eviction halves both the copy and the output DMA;
                    # in the drained tail (no exps in flight) ACT shares it.
                    if tail and nh == 0:
                        nc.scalar.copy(osb[:], pso[:])
                    else:
                        nc.vector.tensor_copy(osb[:], pso[:])
                    nc.sync.dma_start(
                        out_d[it * P:(it + 1) * P, nh * QCH:(nh + 1) * QCH], osb[:]
                    )
                    yield

        def drain(g):
            for _ in g:
                pass

        # normalize: the denominator arrives for free in AV row 64 (ones
        # column of vv).  Per head: evict it, approx-reciprocal, broadcast
        # to 64 partitions via a tiny matmul (bf16 view of the fp32
        # reciprocal -- high half-words -- keeps the matmul off the 4-pass
        # fp32 path), then one fused PSUM-read multiply into aT.
        def emit_norm(ps_acc, den_acc, c, pair):
            dbc = ps_main.tile([P, QCH], FP32, tag="bc", name="dbc")
            for hh in range(2):
                lo = hh * 64
                nc.tensor.matmul(
                    dbc[lo:lo + 64, :],
                    lhsT=ones_bc[:, 0:64],
                    rhs=den_acc[:, hh * QCH:(hh + 1) * QCH],
                    start=True,
                    stop=True,
                )
            dbs = bcp.tile([P, QCH], FP32, tag="dbs", name="dbs")
            nc.vector.tensor_copy(dbs[:], dbc[:])
            rbc = bcp.tile([P, QCH], FP32, tag="rbc", name="rbc")
            nc.vector.reciprocal_approx_fast(rbc[:], dbs[:])
            for hh in range(2):
                lo = hh * 64
                nc.vector.tensor_mul(
                    aT[lo:lo + 64, pair * N + c * QCH: pair * N + (c + 1) * QCH],
                    ps_acc[hh][lo:lo + 64, :],
                    rbc[lo:lo + 64, :],
                )

        def emit_attention_chunk(c, filler=None):
            kjs = _active_kj(c)
            last = len(kjs) - 1
            for pair in range(2):
                ps_acc = [None, None]
                den_acc = None
                pending = []  # (p_tile, i) exp'd tiles not yet fed to AV

                def trim(t):
                    # diagonal tile dt: query columns < 128*dt are fully
                    # causally masked -- skip streaming them.  Not for
                    # chunk 0, whose first scores-psum use would otherwise
                    # leave uninitialized PSUM under the exp (NaN*0=NaN).
                    if c > 0 and 4 * c <= t < 4 * c + 4:
                        return P * (t - 4 * c)
                    return 0

                def do_av(pt, i):
                    # both heads via 128x64 column tiling: head hh lands on
                    # PSUM partitions hh*64..hh*64+63 of its own bank
                    # (separate banks -- the whole-bank has_written clear of
                    # start=True must not race the other head's
                    # accumulation).
                    t = kjs[i]
                    off = trim(t)
                    for hh in range(2):
                        h = pair * 2 + hh
                        lo = hh * 64
                        nc.tensor.matmul(
                            ps_acc[hh][lo:lo + 64, off:QCH],
                            lhsT=vv[:, t * GC + h * 64: t * GC + (h + 1) * 64],
                            rhs=pt[:, hh * QCH + off:(hh + 1) * QCH],
                            start=(i == 0),
                            stop=(i == last),
                        )

                for i, t in enumerate(kjs):
                    # both heads' scores into one 2-bank psum tile
                    off = trim(t)
                    pss = ps_s.tile([P, 2 * QCH], FP32, tag="s", name="pss")
                    for hh in range(2):
                        lo, hi = hh * 64, hh * 64 + 64
                        nc.tensor.matmul(
                            pss[:, hh * QCH + off:(hh + 1) * QCH],
                            lhsT=kT[lo:hi, pair * NM + t * P: pair * NM + (t + 1) * P],
                            rhs=qT[lo:hi, pair * N + c * QCH + off:
                                   pair * N + (c + 1) * QCH],
                            start=True,
                            stop=True,
                        )
                    pt = pexp.tile([P, 2 * QCH], BF16, tag="p", name="pt")
                    nc.scalar.activation(
                        pt[:], pss[:], mybir.ActivationFunctionType.Exp
                    )
                    if 4 * c <= t < 4 * c + 4:  # diagonal tile: causal mask
                        dt = t - 4 * c
                        for hh in range(2):
                            nc.vector.tensor_mul(
                                pt[:, hh * QCH:(hh + 1) * QCH],
                                pt[:, hh * QCH:(hh + 1) * QCH],
                                mks[:, dt * QCH:(dt + 1) * QCH],
                            )
                    # softmax denominator: elementwise accumulate the exp'd
                    # tiles on DVE (the cross-key reduction happens in one
                    # reduce+broadcast matmul per pair at pair end).
                    # Offloading this to GpSimd compute contends for SBUF
                    # ports and slows every engine ~25%; SWDGE dma-accum
                    # serializes on its completion chain; the M=65 ones-row
                    # variant was also measured slower end-to-end.
                    if i == 0:
                        den_acc = bcp.tile(
                            [P, 2 * QCH], BF16, tag="dacc", bufs=2, name="dacc"
                        )
                        nc.vector.tensor_copy(den_acc[:], pt[:])
                    else:
                        nc.vector.tensor_add(den_acc[:], den_acc[:], pt[:])
                    pending.append((pt, i))
                    if i == 0:
                        ps_acc[0] = ps_av.tile([P, QCH], FP32, tag="av", name="av0")
                        ps_acc[1] = ps_av.tile([P, QCH], FP32, tag="av", name="av1")
                    while len(pending) > 1:
                        do_av(*pending.pop(0))
                    if filler is not None:
                        next(filler, None)
                while pending:
                    do_av(*pending.pop(0))
                emit_norm(ps_acc, den_acc, c, pair)

        # ---- emission: prologue projections ordered so pair-0's scores
        # unblock first (mt=0 q/k), then attention chunks with the next
        # chunk's projections and the previous chunk's out-projection
        # drip-fed into the PE queue as filler between attention rounds ----
        drain(gen_qT_group(0, 0))
        drain(gen_kT_group(0, 0))
        drain(gen_qT_group(1, 0))
        drain(gen_kT_group(1, 0))
        drain(gen_v_pair(0))
        drain(gen_v_pair(2))
        for c2 in (4, 5):
            for mt in range(2):
                drain(gen_kT_group(mt, c2))
        for t0 in range(NSELF, NKJ, 2):
            drain(gen_v_pair(t0))
        for c in range(NQC):
            work = []
            if c < NQC - 1:
                c1 = c + 1
                work += [
                    gen_qT_group(0, c1), gen_kT_group(0, c1),
                    gen_qT_group(1, c1), gen_kT_group(1, c1),
                    gen_v_pair(4 * c1), gen_v_pair(4 * c1 + 2),
                ]
            if c > 0:
                work.append(gen_outproj_chunk(c - 1))
            filler = itertools.chain(*work)
            emit_attention_chunk(c, filler)
            drain(filler)
        # keep the PE clock warm through the last normalize wait so the
        # final out-projection doesn't run at the HAM-throttled 1.2 GHz
        for _ in range(8):
            nc.tensor.matmul(
                ps_w[:], lhsT=wrm[:, 0:P], rhs=wrm[:], start=True, stop=True,
            )
        drain(gen_outproj_chunk(NQC - 1, tail=True))

    nc.compile()
    return nc


_CACHE: dict = {}


def _module(biased: bool):
    if biased not in _CACHE:
        _CACHE[biased] = _build_module(biased)
    return _CACHE[biased]


def _mask_tiles():
    t = np.arange(4)[:, None, None]
    p = np.arange(P)[None, :, None]
    q = np.arange(QCH)[None, None, :]
    return (p + P * t <= q).astype(BF16NP).reshape(4 * P, QCH)


def kernel(x, context, Wq, bq, Wkv, bkv, Wo, bo, mask, context_mask):
    assert bool(np.all(mask)) and bool(np.all(context_mask)), (
        "only all-true padding masks are supported"
    )
    x = np.asarray(x, np.float32)
    context = np.asarray(context, np.float32)
    Wq, bq = np.asarray(Wq, np.float32), np.asarray(bq, np.float32)
    Wkv, bkv = np.asarray(Wkv, np.float32), np.asarray(bkv, np.float32)
    Wo, bo = np.asarray(Wo, np.float32), np.asarray(bo, np.float32)

    biased = bool(np.any(bq) or np.any(bkv))
    nc = _module(biased)

    msk = _mask_tiles()
    xkvT = [
        np.ascontiguousarray(
            np.concatenate([x[b], context[b]], axis=0).T.astype(BF16NP)
        )
        for b in range(B)
    ]
    in_maps = []
    for core in range(NCORES):
        b, g = divmod(core, GROUPS)
        cols = slice(g * GC, (g + 1) * GC)
        im = {
            "xkvT": xkvT[b],
            "wq": (Wq[:, cols] * SCALE).astype(BF16NP),
            "wk": Wkv[:, cols].astype(BF16NP),
            "wv": Wkv[:, D + g * GC: D + (g + 1) * GC].astype(BF16NP),
            "wo": np.ascontiguousarray(Wo[cols, :]).astype(BF16NP),
            "msk": msk,
        }
        if biased:
            im["bq"] = (bq[cols] * SCALE).astype(BF16NP).reshape(1, GC)
            im["bk"] = bkv[cols].astype(BF16NP).reshape(1, GC)
            im["bv"] = bkv[D + g * GC: D + (g + 1) * GC].astype(BF16NP).reshape(1, GC)
        in_maps.append(im)

    try:
        res = run_bass_kernel_spmd(nc, in_maps, core_ids=list(range(NCORES)))
    except ModuleNotFoundError:
        # BASS_TRACE set but the NTFF profiling hook isn't available in this
        # environment -- rerun with tracing hard-disabled.
        os.environ["BASS_NEVER_TRACE"] = "1"
        res = run_bass_kernel_spmd(nc, in_maps, core_ids=list(range(NCORES)))
    kernel.last_results = res
    out = np.zeros((B, N, D), np.float32)
    for core in range(NCORES):
        b = core // GROUPS
        out[b] += np.asarray(res.results[core]["out"], dtype=np.float32)
    out += bo
    return out



# revision 88
# speedup vs baseline: 1.0213x; 1.0072x over previous
"""Trainium2 Bass kernel for DecoderAttention (b=2, n=2048, m=1024, d=1024, h=16).

Sharding: 8 cores = 2 (batch) x 4 (head groups of 4 heads).  Each core:
  - projects q/k/v for its 4 heads from x|context (pre-transposed on host),
  - runs causal flash attention in scores-transposed layout [kj, qi]
    (softmax without max subtraction -- scores are bounded; causally masked
    entries multiply to exactly 0 after exp, matching exp(-50000)),
  - computes its partial out-projection  attn_out_g @ Wo[rows_g]  [2048, 1024]
    emitted as bf16.
Host sums the 4 head-group partials per batch (the "all-reduce") and adds bo.

All matmuls run in bf16 with f32 PSUM accumulation (~0.4% rel err; fp8 was
measured at 5-7% end-to-end on the host and is ruled out).

Schedule notes (measured on HW, 354us -> ~253us):
  - The attention body is TensorE-bound at ~1.4us per key-tile round:
    score pair (row-tiled 64x128, truly concurrent: one qT stream feeds
    both sub-arrays) + AV pair (column-tiled 128x64; serialized anyway
    because both heads stream different pt columns through the one rhs
    port) + ~2 drip-fed projection matmuls as filler.
  - exp on ScalarE (144 x ~1.1us) is the close second; scores psum is
    double-buffered so exps run back-to-back except where the PE falls
    behind.
  - The softmax denominator is accumulated elementwise on DVE in bf16 and
    reduced+broadcast by one column-tiled ones-matmul pair per head-pair,
    then reciprocal_approx_fast (the precise DVE reciprocal costs 3.3us
    per tile and had been 53us of the baseline).  GpSimd offload of the
    accumulation poisons SBUF ports (-25% on every engine); SWDGE
    dma-accumulate serializes on its completion chain (2x slower overall).
  - Input DMAs are issued in consumption order (q/k weights, first query
    chunk columns, context columns, rest) and dummy matmuls warm the HAM
    clock gate (1.2 -> 2.4 GHz) during the DMA wait; more dummies keep it
    warm into the final out-projection.
  - Diagonal-tile score/AV matmuls skip the fully-masked leading query
    columns (128*dt) for chunks > 0; chunk 0 streams full width because
    its first scores-psum use would otherwise expose uninitialized PSUM
    to the exp (NaN * mask-zero = NaN).
"""

import os

# The neuron/axon jax backend must be discoverable for the PJRT execution
# path; a JAX_PLATFORMS=cpu pin (used when running the jax reference) would
# hide the trn2 devices from this process.
if os.environ.get("JAX_PLATFORMS", "").strip().lower() == "cpu":
    del os.environ["JAX_PLATFORMS"]

import itertools
from contextlib import ExitStack

import ml_dtypes
import numpy as np

import concourse.bass as bass
import concourse.tile as tile
from concourse import bacc, mybir
from concourse.bass_utils import run_bass_kernel_spmd

B, N, M, D = 2, 2048, 1024, 1024
H, DH = 16, 64
NM = N + M          # 3072 keys (self + context)
GROUPS = 4          # head groups; 4 heads = 256 cols per group
GC = 256            # columns per head group
NCORES = 8
SCALE = DH ** -0.5
P = 128
KT = D // P         # 8 contraction tiles over d
QCH = 512           # query-chunk width
NQC = N // QCH      # 4 query chunks
NKJ = NM // P       # 24 key tiles
NSELF = N // P      # 16 self key tiles
FP32 = mybir.dt.float32
F32R = mybir.dt.float32r
BF16 = mybir.dt.bfloat16
BF16NP = ml_dtypes.bfloat16


def _active_kj(c):
    """Key tiles with any unmasked entry for query chunk c (512 queries)."""
    return list(range(0, 4 * c + 4)) + list(range(NSELF, NKJ))


def _build_module(biased: bool):
    nc = bacc.Bacc(
        "TRN2",
        target_bir_lowering=False,
        debug=False,
        enable_asserts=False,
        num_devices=NCORES,
    )
    xkvT_d = nc.dram_tensor("xkvT", [D, NM], BF16, kind="ExternalInput").ap()
    wq_d = nc.dram_tensor("wq", [D, GC], BF16, kind="ExternalInput").ap()
    wk_d = nc.dram_tensor("wk", [D, GC], BF16, kind="ExternalInput").ap()
    wv_d = nc.dram_tensor("wv", [D, GC], BF16, kind="ExternalInput").ap()
    wo_d = nc.dram_tensor("wo", [GC, D], BF16, kind="ExternalInput").ap()
    msk_d = nc.dram_tensor("msk", [4 * P, QCH], BF16, kind="ExternalInput").ap()
    if biased:
        bq_d = nc.dram_tensor("bq", [1, GC], BF16, kind="ExternalInput").ap()
        bk_d = nc.dram_tensor("bk", [1, GC], BF16, kind="ExternalInput").ap()
        bv_d = nc.dram_tensor("bv", [1, GC], BF16, kind="ExternalInput").ap()
    out_d = nc.dram_tensor("out", [N, D], BF16, kind="ExternalOutput").ap()

    with tile.TileContext(nc) as tc, ExitStack() as ctx:
        const = ctx.enter_context(tc.tile_pool(name="const", bufs=1))
        pexp = ctx.enter_context(tc.tile_pool(name="pexp", bufs=12))
        bcp = ctx.enter_context(tc.tile_pool(name="bcp", bufs=3))
        # PSUM budget: 8 banks = proj(1) + bc(1) + scores(2x2) + av(2)
        ps_main = ctx.enter_context(tc.tile_pool(name="ps_main", bufs=1, space="PSUM"))
        ps_s = ctx.enter_context(tc.tile_pool(name="ps_s", bufs=2, space="PSUM"))
        ps_av = ctx.enter_context(tc.tile_pool(name="ps_av", bufs=2, space="PSUM"))

        # ---- persistent SBUF tensors (column-concatenated k-tiles) ----
        xk = const.tile([P, KT * NM], BF16)          # xkvT: 8 tiles of [128, 3072]
        wqs = const.tile([P, KT * GC], BF16)
        wks = const.tile([P, KT * GC], BF16)
        wvs = const.tile([P, KT * GC], BF16)
        wos = const.tile([P, 2 * D], BF16)           # Wo rows: 2 tiles of [128, 1024]
        mks = const.tile([P, 4 * QCH], BF16)         # 4 diagonal mask tiles
        qT = const.tile([P, 2 * N], BF16)            # [head-pair cols, qi]
        kT = const.tile([P, 2 * NM], BF16)           # [head-pair cols, kj]
        vv = const.tile([P, NKJ * GC], BF16)         # per kj tile: 4 heads x 64
        aT = const.tile([P, 2 * N], BF16)            # attn_out^T, 2 k-tiles
        ones_bc = const.tile([P, 64], BF16)          # all-ones: den reduce+broadcast
        if biased:
            bq_s = const.tile([1, GC], BF16)
            bk_s = const.tile([1, GC], BF16)
            bv_s = const.tile([1, GC], BF16)
            ones_row = const.tile([1, QCH], BF16)
            ones_col = const.tile([1, P], BF16)

        # ---- input DMAs ----
        # One batched DMA per tensor/column-chunk, ordered so the first
        # projections (weights, then x columns for query-chunk 0, then the
        # context columns) unblock compute within a few us instead of after
        # the whole ~9 MB input load.
        xk_v = xk.rearrange("p (kt m) -> p kt m", kt=KT)
        xkvT_v = xkvT_d.rearrange("(kt p) m -> p kt m", p=P)

        def dma_xk(cc):
            nc.sync.dma_start(
                xk_v[:, :, cc * QCH:(cc + 1) * QCH],
                xkvT_v[:, :, cc * QCH:(cc + 1) * QCH],
            )

        # Transfers complete in issue order, so the sequence below is the
        # critical-path priority: q/k weights and the first query-chunk's
        # x columns gate the first scores, then the cross-attention
        # columns, then the rest.  (Finer-grained per-k-tile splits and
        # deferred-injection prologues were tried and measured worse: the
        # wv/mks transfers just land later and stall the first AV/mask
        # rounds instead.)
        nc.sync.dma_start(
            wqs.rearrange("p (kt g) -> p kt g", kt=KT),
            wq_d.rearrange("(kt p) g -> p kt g", p=P),
        )
        nc.sync.dma_start(
            wks.rearrange("p (kt g) -> p kt g", kt=KT),
            wk_d.rearrange("(kt p) g -> p kt g", p=P),
        )
        dma_xk(0)
        dma_xk(4)
        dma_xk(5)
        nc.sync.dma_start(
            wvs.rearrange("p (kt g) -> p kt g", kt=KT),
            wv_d.rearrange("(kt p) g -> p kt g", p=P),
        )
        nc.sync.dma_start(
            mks.rearrange("p (t q) -> p t q", t=4),
            msk_d.rearrange("(t p) q -> p t q", p=P),
        )
        for cc in (1, 2, 3):
            dma_xk(cc)
        nc.sync.dma_start(
            wos.rearrange("p (t d) -> p t d", t=2),
            wo_d.rearrange("(t p) d -> p t d", p=P),
        )
        nc.vector.memset(ones_bc[:], 1.0)
        if biased:
            nc.sync.dma_start(bq_s[:], bq_d[:])
            nc.sync.dma_start(bk_s[:], bk_d[:])
            nc.sync.dma_start(bv_s[:], bv_d[:])
            nc.vector.memset(ones_row[:], 1.0)
            nc.vector.memset(ones_col[:], 1.0)

        # ---- PE warm-up: the HAM clock gate keeps the PE at 1.2 GHz until
        # it has been busy ~3.4us.  Dummy matmuls during the input-DMA wait
        # bring it to 2.4 GHz before the first real projection chain.
        wrm = const.tile([P, QCH], BF16)
        nc.vector.memset(wrm[:], 1.0)
        ps_w = ps_main.tile([P, QCH], FP32, tag="proj", name="warm")
        for _ in range(30):
            nc.tensor.matmul(
                ps_w[:], lhsT=wrm[:, 0:P], rhs=wrm[:], start=True, stop=True,
            )
        # zero the two scores-psum banks once (idle DVE, during the input
        # DMA wait) so chunk 0's diagonal trim below can skip masked
        # columns without exposing uninitialized PSUM to the exp
        for _ in range(2):
            zs = ps_s.tile([P, 2 * QCH], FP32, tag="s", name="zs")
            nc.vector.memset(zs[:], 0.0)

        # ---- emission helpers (generators yield every ~2 matmuls so proj
        # work can be drip-fed into the PE queue between attention rounds
        # without delaying the score matmuls that feed the exp pipeline) ----
        def gen_qT_group(mt, c):
            psq = ps_main.tile([P, QCH], FP32, tag="proj", name="psq")
            for kt in range(KT):
                nc.tensor.matmul(
                    psq[:],
                    lhsT=wqs[:, kt * GC + mt * P: kt * GC + (mt + 1) * P],
                    rhs=xk[:, kt * NM + c * QCH: kt * NM + (c + 1) * QCH],
                    start=(kt == 0),
                    stop=(kt == KT - 1) and not biased,
                )
                if kt % 2 == 1:
                    yield
            if biased:
                nc.tensor.matmul(
                    psq[:], lhsT=bq_s[:, mt * P:(mt + 1) * P], rhs=ones_row[:],
                    start=False, stop=True,
                )
            nc.vector.tensor_copy(
                qT[:, mt * N + c * QCH: mt * N + (c + 1) * QCH], psq[:]
            )
            yield

        def gen_kT_group(mt, c2):
            psk = ps_main.tile([P, QCH], FP32, tag="proj", name="psk")
            for kt in range(KT):
                nc.tensor.matmul(
                    psk[:],
                    lhsT=wks[:, kt * GC + mt * P: kt * GC + (mt + 1) * P],
                    rhs=xk[:, kt * NM + c2 * QCH: kt * NM + (c2 + 1) * QCH],
                    start=(kt == 0),
                    stop=(kt == KT - 1) and not biased,
                )
                if kt % 2 == 1:
                    yield
            if biased:
                nc.tensor.matmul(
                    psk[:], lhsT=bk_s[:, mt * P:(mt + 1) * P], rhs=ones_row[:],
                    start=False, stop=True,
                )
            nc.vector.tensor_copy(
                kT[:, mt * NM + c2 * QCH: mt * NM + (c2 + 1) * QCH], psk[:]
            )
            yield

        def gen_v_pair(t0):
            # two key-tiles' V side by side in one psum bank -> one evict.
            # start=True only on the very first matmul: its whole-bank
            # has_written clear covers both halves (same engine, serial).
            psv = ps_main.tile([P, 2 * GC], FP32, tag="proj", name="psv")
            for kt in range(KT):
                for j in range(2):
                    nc.tensor.matmul(
                        psv[:, j * GC:(j + 1) * GC],
                        lhsT=xk[:, kt * NM + (t0 + j) * P: kt * NM + (t0 + j + 1) * P],
                        rhs=wvs[:, kt * GC:(kt + 1) * GC],
                        start=(kt == 0 and j == 0),
                        stop=(kt == KT - 1) and not biased,
                    )
                yield
            if biased:
                for j in range(2):
                    nc.tensor.matmul(
                        psv[:, j * GC:(j + 1) * GC], lhsT=ones_col[:], rhs=bv_s[:],
                        start=False, stop=True,
                    )
            nc.vector.tensor_copy(vv[:, t0 * GC:(t0 + 2) * GC], psv[:])
            yield

        def gen_outproj_chunk(c, tail=False):
            for it in range(4 * c, 4 * c + 4):
                for nh in range(2):
                    pso = ps_main.tile([P, QCH], FP32, tag="proj", name="pso")
                    for kt in range(2):
                        nc.tensor.matmul(
                            pso[:],
                            lhsT=aT[:, kt * N + it * P: kt * N + (it + 1) * P],
                            rhs=wos[:, kt * D + nh * QCH: kt * D + (nh + 1) * QCH],
                            start=(kt == 0),
                            stop=(kt == 1),
                        )
                    osb = pexp.tile([P, QCH], BF16, tag="osb", bufs=3, name="osb")
                    # bf16 eviction halves both the copy and the output DMA;
                    # in the drained tail (no exps in flight) ACT shares it.
                    if tail and nh == 0:
                        nc.scalar.copy(osb[:], pso[:])
                    else:
                        nc.vector.tensor_copy(osb[:], pso[:])
                    nc.sync.dma_start(
                        out_d[it * P:(it + 1) * P, nh * QCH:(nh + 1) * QCH], osb[:]
                    )
                    yield

        def drain(g):
            for _ in g:
                pass

        # normalize: the denominator arrives for free in AV row 64 (ones
        # column of vv).  Per head: evict it, approx-reciprocal, broadcast
        # to 64 partitions via a tiny matmul (bf16 view of the fp32
        # reciprocal -- high half-words -- keeps the matmul off the 4-pass
        # fp32 path), then one fused PSUM-read multiply into aT.
        def emit_norm(ps_acc, den_acc, c, pair):
            dbc = ps_main.tile([P, QCH], FP32, tag="bc", name="dbc")
            for hh in range(2):
                lo = hh * 64
                nc.tensor.matmul(
                    dbc[lo:lo + 64, :],
                    lhsT=ones_bc[:, 0:64],
                    rhs=den_acc[:, hh * QCH:(hh + 1) * QCH],
                    start=True,
                    stop=True,
                )
            dbs = bcp.tile([P, QCH], FP32, tag="dbs", name="dbs")
            nc.vector.tensor_copy(dbs[:], dbc[:])
            rbc = bcp.tile([P, QCH], FP32, tag="rbc", name="rbc")
            nc.vector.reciprocal_approx_fast(rbc[:], dbs[:])
            for hh in range(2):
                lo = hh * 64
                nc.vector.tensor_mul(
                    aT[lo:lo + 64, pair * N + c * QCH: pair * N + (c + 1) * QCH],
                    ps_acc[hh][lo:lo + 64, :],
                    rbc[lo:lo + 64, :],
                )

        def emit_attention_chunk(c, filler=None):
            kjs = _active_kj(c)
            last = len(kjs) - 1
            for pair in range(2):
                ps_acc = [None, None]
                den_acc = None
                pending = []  # (p_tile, i) exp'd tiles not yet fed to AV

                def trim(t):
                    # diagonal tile dt: query columns < 128*dt are fully
                    # causally masked -- skip streaming them.  The stale
                    # PSUM left under the exp is finite (old scores, or the
                    # startup memset for the first two uses), and the mask
                    # multiply zeroes exp(stale) before the denominator add.
                    if 4 * c <= t < 4 * c + 4:
                        return P * (t - 4 * c)
                    return 0

                def do_av(pt, i):
                    # both heads via 128x64 column tiling: head hh lands on
                    # PSUM partitions hh*64..hh*64+63 of its own bank
                    # (separate banks -- the whole-bank has_written clear of
                    # start=True must not race the other head's
                    # accumulation).
                    t = kjs[i]
                    off = trim(t)
                    for hh in range(2):
                        h = pair * 2 + hh
                        lo = hh * 64
                        nc.tensor.matmul(
                            ps_acc[hh][lo:lo + 64, off:QCH],
                            lhsT=vv[:, t * GC + h * 64: t * GC + (h + 1) * 64],
                            rhs=pt[:, hh * QCH + off:(hh + 1) * QCH],
                            start=(i == 0),
                            stop=(i == last),
                        )

                for i, t in enumerate(kjs):
                    # both heads' scores into one 2-bank psum tile
                    off = trim(t)
                    pss = ps_s.tile([P, 2 * QCH], FP32, tag="s", name="pss")
                    for hh in range(2):
                        lo, hi = hh * 64, hh * 64 + 64
                        nc.tensor.matmul(
                            pss[:, hh * QCH + off:(hh + 1) * QCH],
                            lhsT=kT[lo:hi, pair * NM + t * P: pair * NM + (t + 1) * P],
                            rhs=qT[lo:hi, pair * N + c * QCH + off:
                                   pair * N + (c + 1) * QCH],
                            start=True,
                            stop=True,
                        )
                    pt = pexp.tile([P, 2 * QCH], BF16, tag="p", name="pt")
                    nc.scalar.activation(
                        pt[:], pss[:], mybir.ActivationFunctionType.Exp
                    )
                    if 4 * c <= t < 4 * c + 4:  # diagonal tile: causal mask
                        dt = t - 4 * c
                        for hh in range(2):
                            nc.vector.tensor_mul(
                                pt[:, hh * QCH:(hh + 1) * QCH],
                                pt[:, hh * QCH:(hh + 1) * QCH],
                                mks[:, dt * QCH:(dt + 1) * QCH],
                            )
                    # softmax denominator: elementwise accumulate the exp'd
                    # tiles on DVE (the cross-key reduction happens in one
                    # reduce+broadcast matmul per pair at pair end).
                    # Offloading this to GpSimd compute contends for SBUF
                    # ports and slows every engine ~25%; SWDGE dma-accum
                    # serializes on its completion chain; the M=65 ones-row
                    # variant was also measured slower end-to-end.
                    if i == 0:
                        den_acc = bcp.tile(
                            [P, 2 * QCH], BF16, tag="dacc", bufs=2, name="dacc"
                        )
                        nc.vector.tensor_copy(den_acc[:], pt[:])
                    else:
                        nc.vector.tensor_add(den_acc[:], den_acc[:], pt[:])
                    pending.append((pt, i))
                    if i == 0:
                        ps_acc[0] = ps_av.tile([P, QCH], FP32, tag="av", name="av0")
                        ps_acc[1] = ps_av.tile([P, QCH], FP32, tag="av", name="av1")
                    while len(pending) > 1:
                        do_av(*pending.pop(0))
                    if filler is not None:
                        next(filler, None)
                while pending:
                    do_av(*pending.pop(0))
                emit_norm(ps_acc, den_acc, c, pair)

        # ---- emission: prologue projections ordered so pair-0's scores
        # unblock first (mt=0 q/k), then attention chunks with the next
        # chunk's projections and the previous chunk's out-projection
        # drip-fed into the PE queue as filler between attention rounds ----
        drain(gen_qT_group(0, 0))
        drain(gen_kT_group(0, 0))
        drain(gen_qT_group(1, 0))
        drain(gen_kT_group(1, 0))
        drain(gen_v_pair(0))
        drain(gen_v_pair(2))
        for c2 in (4, 5):
            for mt in range(2):
                drain(gen_kT_group(mt, c2))
        for t0 in range(NSELF, NKJ, 2):
            drain(gen_v_pair(t0))
        for c in range(NQC):
            work = []
            if c < NQC - 1:
                c1 = c + 1
                work += [
                    gen_qT_group(0, c1), gen_kT_group(0, c1),
                    gen_qT_group(1, c1), gen_kT_group(1, c1),
                    gen_v_pair(4 * c1), gen_v_pair(4 * c1 + 2),
                ]
            if c > 0:
                work.append(gen_outproj_chunk(c - 1))
            filler = itertools.chain(*work)
            emit_attention_chunk(c, filler)
            drain(filler)
        # keep the PE clock warm through the last normalize wait so the
        # final out-projection doesn't run at the HAM-throttled 1.2 GHz
        for _ in range(8):
            nc.tensor.matmul(
                ps_w[:], lhsT=wrm[:, 0:P], rhs=wrm[:], start=True, stop=True,
            )
        drain(gen_outproj_chunk(NQC - 1, tail=True))

    nc.compile()
    return nc


_CACHE: dict = {}


def _module(biased: bool):
    if biased not in _CACHE:
        _CACHE[biased] = _build_module(biased)
    return _CACHE[biased]


def _mask_tiles():
    t = np.arange(4)[:, None, None]
    p = np.arange(P)[None, :, None]
    q = np.arange(QCH)[None, None, :]
    return (p + P * t <= q).astype(BF16NP).reshape(4 * P, QCH)


def kernel(x, context, Wq, bq, Wkv, bkv, Wo, bo, mask, context_mask):
    assert bool(np.all(mask)) and bool(np.all(context_mask)), (
        "only all-true padding masks are supported"
    )
    x = np.asarray(x, np.float32)
    context = np.asarray(context, np.float32)
    Wq, bq = np.asarray(Wq, np.float32), np.asarray(bq, np.float32)
    Wkv, bkv = np.asarray(Wkv, np.float32), np.asarray(bkv, np.float32)
    Wo, bo = np.asarray(Wo, np.float32), np.asarray(bo, np.float32)

    biased = bool(np.any(bq) or np.any(bkv))
    nc = _module(biased)

    msk = _mask_tiles()
    xkvT = [
        np.ascontiguousarray(
            np.concatenate([x[b], context[b]], axis=0).T.astype(BF16NP)
        )
        for b in range(B)
    ]
    in_maps = []
    for core in range(NCORES):
        b, g = divmod(core, GROUPS)
        cols = slice(g * GC, (g + 1) * GC)
        im = {
            "xkvT": xkvT[b],
            "wq": (Wq[:, cols] * SCALE).astype(BF16NP),
            "wk": Wkv[:, cols].astype(BF16NP),
            "wv": Wkv[:, D + g * GC: D + (g + 1) * GC].astype(BF16NP),
            "wo": np.ascontiguousarray(Wo[cols, :]).astype(BF16NP),
            "msk": msk,
        }
        if biased:
            im["bq"] = (bq[cols] * SCALE).astype(BF16NP).reshape(1, GC)
            im["bk"] = bkv[cols].astype(BF16NP).reshape(1, GC)
            im["bv"] = bkv[D + g * GC: D + (g + 1) * GC].astype(BF16NP).reshape(1, GC)
        in_maps.append(im)

    try:
        res = run_bass_kernel_spmd(nc, in_maps, core_ids=list(range(NCORES)))
    except ModuleNotFoundError:
        # BASS_TRACE set but the NTFF profiling hook isn't available in this
        # environment -- rerun with tracing hard-disabled.
        os.environ["BASS_NEVER_TRACE"] = "1"
        res = run_bass_kernel_spmd(nc, in_maps, core_ids=list(range(NCORES)))
    kernel.last_results = res
    out = np.zeros((B, N, D), np.float32)
    for core in range(NCORES):
        b = core // GROUPS
        out[b] += np.asarray(res.results[core]["out"], dtype=np.float32)
    out += bo
    return out

